# revision 1
# baseline (speedup 1.0000x reference)
"""Trainium2 Bass kernel for nn_DBGDGMfMRIEncoder (self-contained).

Model: per-ROI 2-layer bidirectional GRU (shared weights) -> multi-head
self-attention over ROIs -> spatial/temporal MLP heads -> output MLP.

Sharding: data-parallel over batch (32 / 8 cores = 4 batches = 800 sequences
per core) for the GRU + attention; the big spatial_fc weight [512, 51200] is
column-sharded across cores with an AllToAll of the attended features and an
AllReduce of the partial s1 products. The tiny temporal head is AllGathered so
every core can finish the output MLP identically.

Layouts (per core):
  - GRU state h: [hidden=128 partitions, seq(batch*roi) cols] fp16
  - gates computed in PSUM via fp16 matmuls (K=hidden / K=1 rank-1 input rows)
  - y0 (layer-0 output) kept in SBUF arenas, fp16
  - roi embeddings roiT: [feature 2x128, 800] fp16 -> attention all on-chip
"""

import math
from contextlib import ExitStack

import numpy as np

import concourse.bass as bass
import concourse.bacc as bacc
import concourse.tile as tile
import concourse.mybir as mybir
import concourse.bass_utils as bass_utils

dt = mybir.dt
AF = mybir.ActivationFunctionType
OP = mybir.AluOpType

N_CORES = 8
B, R, T, H, NH, LAT = 32, 200, 64, 128, 4, 256
D2 = 2 * H
HD = D2 // NH
BN_EPS = 1e-5
B_LOC = B // N_CORES          # 4 batches per core
NS = B_LOC * R                # 800 sequences per core
CHUNKS = [(0, 512), (512, 288)]   # (col offset, width) within the 800 seqs
KBLK = D2 * R // N_CORES      # 6400 spatial-fc columns per core
NKC = KBLK // 128             # 50 k-chunks
RB = R // N_CORES             # 25 ROIs per core-block

_BUILD_CACHE = {}


# --------------------------------------------------------------------------
# device kernel builder
# --------------------------------------------------------------------------

def _declare_inputs(nc):
    f16, f32 = dt.float16, dt.float32
    d = {}
    def inp(name, shape, dty):
        d[name] = nc.dram_tensor(name, list(shape), dty, kind="ExternalInput").ap()
    inp("xT", (T, NS), f16)
    inp("wih0", (2, 3, 128), f16)
    inp("whh0", (2, 3, 128, 128), f16)
    inp("wih1", (2, 3, 2, 128, 128), f16)
    inp("whh1", (2, 3, 128, 128), f16)
    inp("biases", (128, 16), f32)
    inp("ident", (128, 128), f16)
    inp("onesk", (128, 1), f16)
    inp("onesr", (1, 128), f16)
    inp("wqT", (2, 2, 128, 128), f16)
    inp("wkT", (2, 2, 128, 128), f16)
    inp("wvT", (2, 128, 256), f16)
    inp("woT", (2, 2, 128, 128), f16)
    inp("battn", (128, 8), f32)
    inp("sw1", (NKC, 128, 512), f16)
    inp("sb1", (1, 512), f16)
    inp("sw2T", (4, 128, 128), f16)
    inp("sb2", (128, 1), f32)
    inp("tw1T", (2, 2, 128, 128), f16)
    inp("tb1", (128, 2), f32)
    inp("tw2T", (2, 128, 128), f16)
    inp("tb2", (128, 1), f32)
    inp("ow1T", (2, 2, 128, 128), f16)
    inp("ob1", (128, 2), f32)
    inp("ow2T", (2, 2, 128, 128), f16)
    inp("ob2", (128, 2), f32)
    return d


def _load_weights(nc, sb, inp):
    """DMA persistent weights into SBUF tiles; returns dict of tiles."""
    f16, f32 = dt.float16, dt.float32
    w = {}
    def load(name, shape, dty, src_ap):
        t = sb.tile(list(shape), dty, tag=f"w_{name}", name=name)
        nc.sync.dma_start(t[:], src_ap)
        return t
    w["wih0"] = [[load(f"wih0_{d}_{g}", (1, 128), f16, inp["wih0"][d, g:g + 1])
                  for g in range(3)] for d in range(2)]
    w["whh0"] = [[load(f"whh0_{d}_{g}", (128, 128), f16, inp["whh0"][d, g])
                  for g in range(3)] for d in range(2)]
    w["whh1"] = [[load(f"whh1_{d}_{g}", (128, 128), f16, inp["whh1"][d, g])
                  for g in range(3)] for d in range(2)]
    w["wih1"] = [[[load(f"wih1_{d}_{g}_{k}", (128, 128), f16, inp["wih1"][d, g, k])
                   for k in range(2)] for g in range(3)] for d in range(2)]
    w["biases"] = load("biases", (128, 16), f32, inp["biases"][:])
    w["ident"] = load("ident", (128, 128), f16, inp["ident"][:])
    w["onesk"] = load("onesk", (128, 1), f16, inp["onesk"][:])
    w["onesr"] = load("onesr", (1, 128), f16, inp["onesr"][:])
    for nm in ("wqT", "wkT", "woT"):
        w[nm] = [[load(f"{nm}_{m}_{k}", (128, 128), f16, inp[nm][m, k])
                  for k in range(2)] for m in range(2)]
    w["wvT"] = [load(f"wvT_{k}", (128, 256), f16, inp["wvT"][k]) for k in range(2)]
    w["battn"] = load("battn", (128, 8), f32, inp["battn"][:])
    w["sb1"] = load("sb1", (1, 512), f16, inp["sb1"][:])
    w["sw2T"] = [load(f"sw2T_{k}", (128, 128), f16, inp["sw2T"][k]) for k in range(4)]
    w["sb2"] = load("sb2", (128, 1), f32, inp["sb2"][:])
    w["tw1T"] = [[load(f"tw1T_{k}_{m}", (128, 128), f16, inp["tw1T"][k, m])
                  for m in range(2)] for k in range(2)]
    w["tb1"] = load("tb1", (128, 2), f32, inp["tb1"][:])
    w["tw2T"] = [load(f"tw2T_{k}", (128, 128), f16, inp["tw2T"][k]) for k in range(2)]
    w["tb2"] = load("tb2", (128, 1), f32, inp["tb2"][:])
    w["ow1T"] = [[load(f"ow1T_{k}_{m}", (128, 128), f16, inp["ow1T"][k, m])
                  for m in range(2)] for k in range(2)]
    w["ob1"] = load("ob1", (128, 2), f32, inp["ob1"][:])
    w["ow2T"] = [[load(f"ow2T_{k}_{m}", (128, 128), f16, inp["ow2T"][k, m])
                  for m in range(2)] for k in range(2)]
    w["ob2"] = load("ob2", (128, 2), f32, inp["ob2"][:])
    return w


def _emit_scan(nc, tc, w, inp, layer, c0, F, ps, sp, xp, arenas, roiT):
    """One bidirectional GRU layer scan over batch-chunk [c0, c0+F).

    layer 0: inputs from xT (rank-1), writes y0 arenas.
    layer 1: inputs from y0 arenas, writes roiT at final step.
    """
    f16 = dt.float16
    biases = w["biases"]

    hz = sp.tile([128, F], f16, tag="hz", name="hz", bufs=1)
    nc.vector.memset(hz[:], 0.0)

    h_cur = [None, None]  # layer-1 hidden state tiles

    for t in range(T):
        for d in range(2):
            p = t if d == 0 else T - 1 - t       # sequence position computed
            bc = (layer * 2 + d) * 4             # bias column base
            ps_rz = ps.tile([128, 1024], dt.float32, tag="rz", name="ps_rz")
            ps_hn = ps.tile([128, 512], dt.float32, tag="hn", name="ps_hn")
            ps_in = ps.tile([128, 512], dt.float32, tag="in", name="ps_in")

            if layer == 0:
                h_prev = hz[:] if t == 0 else (
                    arenas[d][:, (p - 1 if d == 0 else p + 1) * F:
                              (p - 1 if d == 0 else p + 1) * F + F])
                xt = xp.tile([1, F], f16, tag="xt", name="xt")
                nc.sync.dma_start(xt[:], inp["xT"][p:p + 1, c0:c0 + F])
                # r gate
                nc.tensor.matmul(ps_rz[:, 0:F], w["whh0"][d][0][:], h_prev,
                                 start=True, stop=False)
                nc.tensor.matmul(ps_rz[:, 0:F], w["wih0"][d][0][:], xt[:],
                                 start=False, stop=True)
                # z gate
                nc.tensor.matmul(ps_rz[:, 512:512 + F], w["whh0"][d][1][:], h_prev,
                                 start=True, stop=False)
                nc.tensor.matmul(ps_rz[:, 512:512 + F], w["wih0"][d][1][:], xt[:],
                                 start=False, stop=True)
                # hn / i_n
                nc.tensor.matmul(ps_hn[:, 0:F], w["whh0"][d][2][:], h_prev,
                                 start=True, stop=True)
                nc.tensor.matmul(ps_in[:, 0:F], w["wih0"][d][2][:], xt[:],
                                 start=True, stop=False)
            else:
                h_prev = hz[:] if t == 0 else h_cur[d]
                yf = arenas[0][:, p * F:p * F + F]
                yb = arenas[1][:, p * F:p * F + F]
                for g, dst in ((0, ps_rz[:, 0:F]), (1, ps_rz[:, 512:512 + F])):
                    nc.tensor.matmul(dst, w["wih1"][d][g][0][:], yf, start=True, stop=False)
                    nc.tensor.matmul(dst, w["wih1"][d][g][1][:], yb, start=False, stop=False)
                    nc.tensor.matmul(dst, w["whh1"][d][g][:], h_prev, start=False, stop=True)
                nc.tensor.matmul(ps_in[:, 0:F], w["wih1"][d][2][0][:], yf,
                                 start=True, stop=False)
                nc.tensor.matmul(ps_in[:, 0:F], w["wih1"][d][2][1][:], yb,
                                 start=False, stop=False)
                nc.tensor.matmul(ps_hn[:, 0:F], w["whh1"][d][2][:], h_prev,
                                 start=True, stop=True)

            r_sb = sp.tile([128, F], f16, tag="r", name="r_sb")
            z_sb = sp.tile([128, F], f16, tag="z", name="z_sb")
            nc.scalar.activation(r_sb[:], ps_rz[:, 0:F], AF.Sigmoid,
                                 bias=biases[:, bc + 0:bc + 1])
            nc.scalar.activation(z_sb[:], ps_rz[:, 512:512 + F], AF.Sigmoid,
                                 bias=biases[:, bc + 1:bc + 2])
            # q = (hn + b_hh_n) * r
            q_sb = sp.tile([128, F], f16, tag="q", name="q_sb")
            nc.vector.scalar_tensor_tensor(q_sb[:], ps_hn[:, 0:F],
                                           biases[:, bc + 2:bc + 3], r_sb[:],
                                           OP.add, OP.mult)
            # ps_in += q  (identity matmul accumulate)
            nc.tensor.matmul(ps_in[:, 0:F], w["ident"][:], q_sb[:],
                             start=False, stop=True)
            n_sb = sp.tile([128, F], f16, tag="n", name="n_sb")
            nc.scalar.activation(n_sb[:], ps_in[:, 0:F], AF.Tanh,
                                 bias=biases[:, bc + 3:bc + 4])
            # h' = n + z*(h_prev - n)
            u_sb = sp.tile([128, F], f16, tag="u", name="u_sb")
            v_sb = sp.tile([128, F], f16, tag="v", name="v_sb")
            nc.vector.tensor_tensor(u_sb[:], h_prev, n_sb[:], OP.subtract)
            nc.vector.tensor_tensor(v_sb[:], z_sb[:], u_sb[:], OP.mult)
            if layer == 0:
                dest = arenas[d][:, p * F:p * F + F]
                nc.vector.tensor_tensor(dest, n_sb[:], v_sb[:], OP.add)
            elif t == T - 1:
                dest = roiT[d][:, c0:c0 + F]
                nc.vector.tensor_tensor(dest, n_sb[:], v_sb[:], OP.add)
            else:
                hn_t = sp.tile([128, F], f16, tag=f"h1_{d}", name=f"h1_{d}")
                nc.vector.tensor_tensor(hn_t[:], n_sb[:], v_sb[:], OP.add)
                h_cur[d] = hn_t[:]


def _emit_attention(nc, tc, w, b, roiT, attT, pooled, aps, asb):
    """Attention for local batch index b (cols 200b..200b+200 of roiT)."""
    f16 = dt.float16
    cols = slice(R * b, R * b + R)
    battn = w["battn"]
    kw = [128, R - 128]                      # k chunks over 200 ROIs

    qt, kt = [], []
    for which, wt, bcol in (("q", w["wqT"], 0), ("k", w["wkT"], 2)):
        for m in range(2):
            psq = aps.tile([128, R], dt.float32, tag="qk_ps", name="psq", bufs=1)
            nc.tensor.matmul(psq[:], wt[m][0][:], roiT[0][:, cols], start=True, stop=False)
            nc.tensor.matmul(psq[:], wt[m][1][:], roiT[1][:, cols], start=False, stop=True)
            sbq = asb.tile([128, R], f16, tag="qk_sb", name="sbq", bufs=4)
            nc.scalar.activation(sbq[:], psq[:], AF.Identity,
                                 bias=battn[:, bcol + m:bcol + m + 1])
            (qt if which == "q" else kt).append(sbq)

    vkd = []
    for kc in range(2):
        k0, kn = 128 * kc, kw[kc]
        psv = aps.tile([128, 256], dt.float32, tag="v_ps", name="psv")
        nc.tensor.matmul(psv[0:kn, :], roiT[0][:, R * b + k0:R * b + k0 + kn],
                         w["wvT"][0][:], start=True, stop=False)
        nc.tensor.matmul(psv[0:kn, :], roiT[1][:, R * b + k0:R * b + k0 + kn],
                         w["wvT"][1][:], start=False, stop=True)
        sbv = asb.tile([128, 256], f16, tag="v_sb", name="sbv", bufs=2)
        nc.scalar.activation(sbv[0:kn, :], psv[0:kn, :], AF.Copy)
        vkd.append(sbv)

    ps_av = [aps.tile([128, R], dt.float32, tag=f"av_ps{m2}", name=f"ps_av{m2}", bufs=1) for m2 in range(2)]
    for hd in range(NH):
        m, off = hd // 2, 64 * (hd % 2)
        eT = []
        ps_den = aps.tile([1, R], dt.float32, tag="den_ps", name="ps_den", bufs=1)
        for kc in range(2):
            kn = kw[kc]
            ps_s = aps.tile([128, R], dt.float32, tag=f"sc_ps{kc}", name=f"ps_s{kc}", bufs=1)
            nc.tensor.matmul(ps_s[0:kn, :],
                             kt[m][off:off + 64, 128 * kc:128 * kc + kn],
                             qt[m][off:off + 64, :], start=True, stop=True)
            e_sb = asb.tile([128, R], f16, tag="e_sb", name="e_sb", bufs=2)
            nc.scalar.activation(e_sb[0:kn, :], ps_s[0:kn, :], AF.Exp,
                                 scale=1.0 / math.sqrt(HD))
            nc.tensor.matmul(ps_den[:], w["onesk"][0:kn, :], e_sb[0:kn, :],
                             start=(kc == 0), stop=(kc == 1))
            eT.append(e_sb)
        den_r = asb.tile([1, R], dt.float32, tag="den_sb", name="den_r")
        nc.vector.reciprocal(den_r[:], ps_den[:])
        den16 = asb.tile([1, R], f16, tag="den16", name="den16", bufs=2)
        nc.vector.tensor_copy(den16[:], den_r[:])
        den_b = aps.tile([128, R], dt.float32, tag="sc_ps0", name="den_b", bufs=1)
        nc.tensor.matmul(den_b[:], w["onesr"][:], den16[:], start=True, stop=True)
        for kc in range(2):
            kn = kw[kc]
            nc.vector.tensor_tensor(eT[kc][0:kn, :], eT[kc][0:kn, :],
                                    den_b[0:kn, :], OP.mult)
            m2 = hd // 2
            nc.tensor.matmul(ps_av[m2][off:off + 64, :],
                             vkd[kc][0:kn, 64 * hd:64 * hd + 64],
                             eT[kc][0:kn, :],
                             start=(kc == 0), stop=(kc == 1))
    av_sb = []
    for m2 in range(2):
        sba = asb.tile([128, R], f16, tag="av_sb", name="sba", bufs=2)
        nc.scalar.activation(sba[:], ps_av[m2][:], AF.Identity,
                             bias=battn[:, 4 + m2:5 + m2])
        av_sb.append(sba)

    for m in range(2):
        ps_at = aps.tile([128, R], dt.float32, tag="at_ps", name="ps_at", bufs=1)
        nc.tensor.matmul(ps_at[:], w["woT"][m][0][:], av_sb[0][:], start=True, stop=False)
        nc.tensor.matmul(ps_at[:], w["woT"][m][1][:], av_sb[1][:], start=False, stop=True)
        nc.scalar.activation(attT[m][:, cols], ps_at[:], AF.Identity,
                             bias=battn[:, 6 + m:7 + m],
                             accum_out=pooled[:, m * B_LOC + b:m * B_LOC + b + 1])


def build_kernel(loop_R=1, debug=False, probe=None):
    key = (loop_R, debug, probe)
    if key in _BUILD_CACHE:
        return _BUILD_CACHE[key]
    f16, f32 = dt.float16, dt.float32
    nc = bacc.Bacc("TRN2", target_bir_lowering=False, debug=False,
                   num_devices=N_CORES)
    inp = _declare_inputs(nc)
    out = nc.dram_tensor("out", [2, 128, B], f32, kind="ExternalOutput").ap()
    dbg = {}
    if debug:
        for nm, shp, dty in (("dbg_roi", [2, 128, NS], f16),
                             ("dbg_att", [2, 128, NS], f16),
                             ("dbg_pooled", [128, 2 * B_LOC], f32),
                             ("dbg_s1", [B, 512], f32),
                             ("dbg_y0f", [128, T * 512], f16),
                             ("dbg_y0b", [128, T * 512], f16)):
            dbg[nm] = nc.dram_tensor(nm, shp, dty, kind="ExternalOutput").ap()

    with tile.TileContext(nc) as tc:
        with ExitStack() as ctx:
            dram = ctx.enter_context(tc.tile_pool(name="dram", bufs=1, space="DRAM"))
            a2a_in = dram.tile([N_CORES, 256, B_LOC, RB], f16, name="a2a_in")
            a2a_out = dram.tile([N_CORES, 256, B_LOC, RB], f16, name="a2a_out")
            ar_in = dram.tile([B, 512], f32, name="ar_in")
            ar_out = dram.tile([B, 512], f32, name="ar_out")
            ag_in = dram.tile([128, B_LOC], f32, name="ag_in")
            ag_out = dram.tile([N_CORES, 128, B_LOC], f32, name="ag_out")

            sb = ctx.enter_context(tc.tile_pool(name="weights", bufs=1))
            w = _load_weights(nc, sb, inp)
            roiT = [sb.tile([128, NS], f16, tag=f"roiT_{d}", name=f"roiT_{d}") for d in range(2)]
            attT = [sb.tile([128, NS], f16, tag=f"attT_{m}", name=f"attT_{m}") for m in range(2)]
            pooled = sb.tile([128, 2 * B_LOC], f32, tag="pooled", name="pooled")

            loop_ctx = tc.For_i(0, loop_R, 1) if loop_R > 1 else None
            if loop_ctx is not None:
                loop_ctx.__enter__()

            # ---- GRU phases, chunk by chunk ----
            chunk_list = CHUNKS[:1] if probe == "c1" else CHUNKS
            for ci, (c0, F) in enumerate(chunk_list):
                with ExitStack() as cctx:
                    yp = cctx.enter_context(
                        tc.tile_pool(name=f"y0_{ci}", bufs=1))
                    ps = cctx.enter_context(
                        tc.tile_pool(name=f"ps_{ci}", bufs=2, space="PSUM"))
                    sp = cctx.enter_context(tc.tile_pool(name=f"sp_{ci}", bufs=3))
                    xp = cctx.enter_context(tc.tile_pool(name=f"xp_{ci}", bufs=4))
                    arenas = [yp.tile([128, T * F], f16, tag=f"y0a_{d}", name=f"y0a_{d}")
                              for d in range(2)]
                    _emit_scan(nc, tc, w, inp, 0, c0, F, ps, sp, xp, arenas, roiT)
                    if debug and ci == 0:
                        nc.sync.dma_start(dbg["dbg_y0f"][:], arenas[0][:])
                        nc.sync.dma_start(dbg["dbg_y0b"][:], arenas[1][:])
                    if probe == "l0":
                        # touch arenas so the scan isn't dead-code eliminated
                        nc.vector.tensor_tensor(roiT[0][:, c0:c0 + F],
                                                arenas[0][:, 0:F],
                                                arenas[1][:, 0:F], OP.add)
                        continue
                    _emit_scan(nc, tc, w, inp, 1, c0, F, ps, sp, xp, arenas, roiT)

            # ---- attention (per local batch) ----
            with ExitStack() as actx:
                aps = actx.enter_context(
                    tc.tile_pool(name="att_ps", bufs=1, space="PSUM"))
                asb = actx.enter_context(tc.tile_pool(name="att_sb", bufs=4))
                if probe not in ("l0", "l1"):
                    for b in range(B_LOC):
                        _emit_attention(nc, tc, w, b, roiT, attT, pooled, aps, asb)
                else:
                    for m in range(2):
                        nc.vector.tensor_copy(attT[m][:], roiT[m][:])
                        nc.vector.memset(pooled[:, 0:2 * B_LOC], 0.0)

                # ship attended features to the AllToAll bounce buffer
                for m in range(2):
                    for b in range(B_LOC):
                        src = attT[m][:, R * b:R * (b + 1)].rearrange(
                            "p (j r) -> p j r", j=N_CORES, r=RB)
                        dst = a2a_in[:, 128 * m:128 * m + 128, b, :] \
                            .rearrange("j p r -> p j r")
                        nc.sync.dma_start(dst, src)

                # temporal head (local batches), then AllGather bounce
                pooled16 = asb.tile([128, 2 * B_LOC], f16, tag="pooled16", name="pooled16", bufs=1)
                nc.vector.tensor_copy(pooled16[:], pooled[:])
                t1_sb = []
                for m in range(2):
                    ps_t = aps.tile([128, B_LOC], f32, tag="at_ps", name="ps_t", bufs=1)
                    for kc in range(2):
                        rhs = pooled16[:, kc * B_LOC:(kc + 1) * B_LOC]
                        nc.tensor.matmul(ps_t[:], w["tw1T"][kc][m][:], rhs,
                                         start=(kc == 0), stop=(kc == 1))
                    sbt = asb.tile([128, B_LOC], f16, tag="t1_sb", name="sbt", bufs=2)
                    nc.scalar.activation(sbt[:], ps_t[:], AF.Relu,
                                         bias=w["tb1"][:, m:m + 1])
                    t1_sb.append(sbt)
                ps_t2 = aps.tile([128, B_LOC], f32, tag="at_ps", name="ps_t2", bufs=1)
                for kc in range(2):
                    nc.tensor.matmul(ps_t2[:], w["tw2T"][kc][:], t1_sb[kc][:],
                                     start=(kc == 0), stop=(kc == 1))
                temporalT = asb.tile([128, B_LOC], f32, tag="tout_sb", name="temporalT", bufs=1)
                nc.scalar.activation(temporalT[:], ps_t2[:], AF.Identity,
                                     bias=w["tb2"][:, 0:1])
                nc.sync.dma_start(ag_in[:], temporalT[:])

            if loop_ctx is not None:
                loop_ctx.__exit__(None, None, None)

            # ---- collectives ----
            if debug:
                for d in range(2):
                    nc.sync.dma_start(dbg["dbg_roi"][d], roiT[d][:])
                    nc.sync.dma_start(dbg["dbg_att"][d], attT[d][:])
                nc.sync.dma_start(dbg["dbg_pooled"][:], pooled[:])
            nc.gpsimd.collective_compute(
                "AllToAll", OP.bypass, replica_groups=[list(range(N_CORES))],
                ins=[a2a_in.opt()], outs=[a2a_out.opt()])
            nc.gpsimd.collective_compute(
                "AllGather", OP.bypass, replica_groups=[list(range(N_CORES))],
                ins=[ag_in.opt()], outs=[ag_out.opt()])

            # ---- spatial fc partial: s1T[b, m] over local k block ----
            with ExitStack() as sctx:
                sps = sctx.enter_context(
                    tc.tile_pool(name="s_ps", bufs=2, space="PSUM"))
                ssb = sctx.enter_context(tc.tile_pool(name="s_sb", bufs=2))
                swp = sctx.enter_context(tc.tile_pool(name="sw_pool", bufs=4))

                flatk = []
                for dm in range(2):
                    fk = ssb.tile([128, N_CORES * B_LOC * RB], f16, tag="flatk", name="fk", bufs=2)
                    for j in range(N_CORES):
                        src = a2a_out[j, 128 * dm:128 * dm + 128, :, :] \
                            .rearrange("p b r -> p (b r)")
                        nc.sync.dma_start(
                            fk[:, B_LOC * RB * j:B_LOC * RB * (j + 1)], src)
                    flatk.append(fk)

                ps_s1 = sps.tile([B, 512], f32, tag="s1_ps", name="ps_s1", bufs=1)
                for kc in range(NKC):
                    r_loc, dm = kc // 2, kc % 2
                    swt = swp.tile([128, 512], f16, tag="swt", name="swt")
                    nc.sync.dma_start(swt[:], inp["sw1"][kc])
                    lhs = flatk[dm][:, :].rearrange(
                        "p (j b r) -> p r (j b)", j=N_CORES, b=B_LOC, r=RB
                    )[:, r_loc:r_loc + 1, :].opt()
                    nc.tensor.matmul(ps_s1[:], lhs, swt[:],
                                     start=(kc == 0), stop=False)
                nc.tensor.matmul(ps_s1[:], w["onesr"][:, 0:B], w["sb1"][:],
                                 start=False, stop=True)
                s1_sb = ssb.tile([B, 512], f32, tag="s1_sb", name="s1_sb", bufs=1)
                nc.scalar.activation(s1_sb[:], ps_s1[:], AF.Copy)
                nc.sync.dma_start(ar_in[:], s1_sb[:])

            nc.gpsimd.collective_compute(
                "AllReduce", OP.add, replica_groups=[list(range(N_CORES))],
                ins=[ar_in.opt()], outs=[ar_out.opt()])

            # ---- tail: s2, concat, output MLP (all 32 batches, redundant) ----
            with ExitStack() as tctx:
                tps = tctx.enter_context(
                    tc.tile_pool(name="tail_ps", bufs=2, space="PSUM"))
                tsb = tctx.enter_context(tc.tile_pool(name="tail_sb", bufs=4))

                s1_all = tsb.tile([B, 512], f32, tag="s1_all", name="s1_all", bufs=1)
                nc.sync.dma_start(s1_all[:], ar_out[:])
                if debug:
                    nc.sync.dma_start(dbg["dbg_s1"][:], s1_all[:])
                s1_relu = tsb.tile([B, 512], f16, tag="s1_relu", name="s1_relu", bufs=1)
                nc.scalar.activation(s1_relu[:], s1_all[:], AF.Relu)
                s1T = []
                for kc in range(4):
                    ps_tr = tps.tile([128, B], dt.float16, tag="tr_ps", name="ps_tr", bufs=2)
                    nc.tensor.transpose(ps_tr[:], s1_relu[:, 128 * kc:128 * kc + 128],
                                        w["ident"][0:B, 0:B])
                    sbt = tsb.tile([128, B], f16, tag="s1T", name="s1T", bufs=4)
                    nc.scalar.activation(sbt[:], ps_tr[:], AF.Copy)
                    s1T.append(sbt)
                ps_s2 = tps.tile([128, B], f32, tag="mm_ps", name="ps_s2", bufs=2)
                for kc in range(4):
                    nc.tensor.matmul(ps_s2[:], w["sw2T"][kc][:], s1T[kc][:],
                                     start=(kc == 0), stop=(kc == 3))
                spatialT = tsb.tile([128, B], f16, tag="spatialT", name="spatialT", bufs=1)
                nc.scalar.activation(spatialT[:], ps_s2[:], AF.Identity,
                                     bias=w["sb2"][:, 0:1])

                temporal32 = tsb.tile([128, B], f16, tag="temporal32", name="temporal32", bufs=1)
                nc.gpsimd.dma_start(
                    temporal32[:].rearrange("p (j b) -> p j b",
                                            j=N_CORES, b=B_LOC),
                    ag_out[:, :, :].rearrange("j p b -> p j b"))

                o1_sb = []
                for m in range(2):
                    ps_o = tps.tile([128, B], f32, tag="mm_ps", name="ps_o", bufs=2)
                    nc.tensor.matmul(ps_o[:], w["ow1T"][0][m][:], temporal32[:],
                                     start=True, stop=False)
                    nc.tensor.matmul(ps_o[:], w["ow1T"][1][m][:], spatialT[:],
                                     start=False, stop=True)
                    sbo = tsb.tile([128, B], f16, tag="o1_sb", name="sbo", bufs=2)
                    nc.scalar.activation(sbo[:], ps_o[:], AF.Relu,
                                         bias=w["ob1"][:, m:m + 1])
                    o1_sb.append(sbo)
                for m in range(2):
                    ps_o = tps.tile([128, B], f32, tag="mm_ps", name="ps_o", bufs=2)
                    nc.tensor.matmul(ps_o[:], w["ow2T"][0][m][:], o1_sb[0][:],
                                     start=True, stop=False)
                    nc.tensor.matmul(ps_o[:], w["ow2T"][1][m][:], o1_sb[1][:],
                                     start=False, stop=True)
                    out_sb = tsb.tile([128, B], f32, tag="out_sb", name="out_sb", bufs=2)
                    nc.scalar.activation(out_sb[:], ps_o[:], AF.Identity,
                                         bias=w["ob2"][:, m:m + 1])
                    nc.sync.dma_start(out[m], out_sb[:])

    nc.compile()
    _BUILD_CACHE[key] = nc
    return nc


# --------------------------------------------------------------------------
# host-side weight preparation
# --------------------------------------------------------------------------

def _prep_inputs(inputs):
    """Build the 8 per-core in_maps from the full-model input dict."""
    f16 = np.float16
    g = {k: np.asarray(v) for k, v in inputs.items()}

    def gate_chunks(a):
        return [a[128 * i:128 * (i + 1)] for i in range(3)]

    shared = {}
    # layer 0 / layer 1 weights, torch gate order (r, z, n)
    wih0 = np.zeros((2, 3, 128), f16)
    whh0 = np.zeros((2, 3, 128, 128), f16)
    wih1 = np.zeros((2, 3, 2, 128, 128), f16)
    whh1 = np.zeros((2, 3, 128, 128), f16)
    biases = np.zeros((128, 16), np.float32)
    for d, sfx in enumerate(("", "r")):
        for gi, (wc, hc) in enumerate(zip(gate_chunks(g[f"w_ih_l0{sfx}"]),
                                          gate_chunks(g[f"w_hh_l0{sfx}"]))):
            wih0[d, gi] = wc[:, 0].astype(f16)
            whh0[d, gi] = hc.T.astype(f16)
        for gi, (wc, hc) in enumerate(zip(gate_chunks(g[f"w_ih_l1{sfx}"]),
                                          gate_chunks(g[f"w_hh_l1{sfx}"]))):
            for kc in range(2):
                wih1[d, gi, kc] = wc[:, 128 * kc:128 * (kc + 1)].T.astype(f16)
            whh1[d, gi] = hc.T.astype(f16)
        for li, lname in enumerate(("l0", "l1")):
            bih = gate_chunks(g[f"b_ih_{lname}{sfx}"])
            bhh = gate_chunks(g[f"b_hh_{lname}{sfx}"])
            bc = (li * 2 + d) * 4
            biases[:, bc + 0] = bih[0] + bhh[0]
            biases[:, bc + 1] = bih[1] + bhh[1]
            biases[:, bc + 2] = bhh[2]
            biases[:, bc + 3] = bih[2]
    shared.update(wih0=wih0, whh0=whh0, wih1=wih1, whh1=whh1, biases=biases)
    shared["ident"] = np.eye(128, dtype=f16)
    shared["onesk"] = np.ones((128, 1), f16)
    shared["onesr"] = np.ones((1, 128), f16)

    def blocksT(wm):
        return np.stack([
            np.stack([wm[128 * m:128 * (m + 1), 128 * k:128 * (k + 1)].T.astype(f16)
                      for k in range(2)]) for m in range(2)])
    shared["wqT"] = blocksT(g["wq"])
    shared["wkT"] = blocksT(g["wk"])
    shared["woT"] = blocksT(g["wo"])
    shared["wvT"] = np.stack([g["wv"].T[128 * k:128 * (k + 1)].astype(f16)
                              for k in range(2)])
    battn = np.zeros((128, 8), np.float32)
    for m in range(2):
        battn[:, 0 + m] = g["bq"][128 * m:128 * (m + 1)]
        battn[:, 2 + m] = g["bk"][128 * m:128 * (m + 1)]
        battn[:, 4 + m] = g["bv"][128 * m:128 * (m + 1)]
        battn[:, 6 + m] = g["bo"][128 * m:128 * (m + 1)]
    shared["battn"] = battn

    shared["sb1"] = (g["s_b1"][None, :] / N_CORES).astype(f16)
    shared["sw2T"] = np.stack([g["s_w2"][:, 128 * k:128 * (k + 1)].T.astype(f16)
                               for k in range(4)])
    shared["sb2"] = g["s_b2"][:, None].astype(np.float32)

    # temporal head with BN folded in (and the 1/R mean scale)
    s = g["bn_g"] / np.sqrt(g["bn_v"] + BN_EPS)
    o = g["bn_b"] - g["bn_m"] * s
    tw1 = g["t_w1"] * (s / R)[None, :]
    tb1 = g["t_w1"] @ o + g["t_b1"]
    shared["tw1T"] = np.stack([
        np.stack([tw1[128 * m:128 * (m + 1), 128 * k:128 * (k + 1)].T.astype(f16)
                  for m in range(2)]) for k in range(2)])
    tb1_a = np.zeros((128, 2), np.float32)
    for m in range(2):
        tb1_a[:, m] = tb1[128 * m:128 * (m + 1)]
    shared["tb1"] = tb1_a
    shared["tw2T"] = np.stack([g["t_w2"][:, 128 * k:128 * (k + 1)].T.astype(f16)
                               for k in range(2)])
    shared["tb2"] = g["t_b2"][:, None].astype(np.float32)

    def blocksT2(wm):   # [kc][m] layout
        return np.stack([
            np.stack([wm[128 * m:128 * (m + 1), 128 * k:128 * (k + 1)].T.astype(f16)
                      for m in range(2)]) for k in range(2)])
    shared["ow1T"] = blocksT2(g["o_w1"])
    shared["ob1"] = np.stack([g["o_b1"][0:128], g["o_b1"][128:256]], axis=1).astype(np.float32)
    shared["ow2T"] = blocksT2(g["o_w2"])
    shared["ob2"] = np.stack([g["o_b2"][0:128], g["o_b2"][128:256]], axis=1).astype(np.float32)

    fmri = g["fmri"]
    in_maps = []
    for c in range(N_CORES):
        m = dict(shared)
        xs = fmri[B_LOC * c:B_LOC * (c + 1)].reshape(NS, T)
        m["xT"] = np.ascontiguousarray(xs.T).astype(f16)
        swc = g["s_w1"][:, KBLK * c:KBLK * (c + 1)]      # [512, 6400]
        m["sw1"] = np.ascontiguousarray(swc.T.reshape(NKC, 128, 512)).astype(f16)
        in_maps.append(m)
    return in_maps


def kernel(**inputs) -> np.ndarray:
    nc = build_kernel(1)
    in_maps = _prep_inputs(inputs)
    res = bass_utils.run_bass_kernel_spmd(nc, in_maps,
                                          core_ids=list(range(N_CORES)))
    o = res.results[0]["out"]          # [2, 128, 32] f32, identical on cores
    z = o.reshape(256, B).T            # -> [32, 256]
    return np.ascontiguousarray(z.astype(np.float32))

